# revision 3
# baseline (speedup 1.0000x reference)
"""Trainium2 Bass kernel for nn_LongTermFeatureBank — single-core version.

Rationale: the module's legacy softmax over dim=0 (batch) makes any
batch-sharded multi-core layout depend on a collective; in the grading
environment the collective entry stalls on the slowest-to-start core,
which dominated the old 8-core kernel's measured time.  This version
runs the full batch on one core with fp16 inputs/weights (f32
accumulation), so the measured span is pure local DMA + compute.

Performance notes:
  * x rows are host-padded from 49 to 50 fp16 elements so every DVE
    access is 4B-aligned — required for the vector engine's 2x packed
    mode on the pooling reduce.
  * PSUM->SBUF drains ride the scalar engine (activation Copy /
    Identity-with-bias), keeping the vector engine free for pooling.
  * LayerNorm mean/rstd are expanded to full [128, R] operands with
    rank-1 matmuls so the normalize runs as flat, contiguous, all-fp16
    tensor_tensor ops (2x mode) instead of strided broadcasts.
"""
import os
import numpy as np
import ml_dtypes

import concourse.bass as bass
import concourse.tile as tile
from concourse import bacc, mybir
from concourse.bass_utils import run_bass_kernel_spmd

F32 = mybir.dt.float32
F16 = mybir.dt.float16
X = mybir.AxisListType.X
ADD = mybir.AluOpType.add
SUB = mybir.AluOpType.subtract
MUL = mybir.AluOpType.mult
EXP = mybir.ActivationFunctionType.Exp
RELU = mybir.ActivationFunctionType.Relu
SQRT = mybir.ActivationFunctionType.Sqrt
COPY = mybir.ActivationFunctionType.Copy
IDENT = mybir.ActivationFunctionType.Identity

B, D1, D2, HW = 128, 512, 16, 49
HWP = 52              # host-padded pooling window (4B alignment for DVE 2x)
NC = 1
R = B * D2            # 2048 rows (b,d2)
NH = R // 128         # 16 h-blocks
NRB = R // 512        # 4 rb-blocks
ISD = float(1.0 / np.sqrt(D1))
NB = 8                # batches per x DMA chunk -> one h-block per chunk
LAST = {}
USE_AFFINE = [True]   # set by make_in_maps from actual ln_g/ln_b values


def _declare(nc):
    ap = {}
    def di(name, shape, dt=F32):
        ap[name] = nc.dram_tensor(name, list(shape), dt, kind="ExternalInput").ap()
    di("xr", (4, 128, B, D2 * HWP), F16)
    for s in range(2):
        di(f"wa{s}", (128, 4, D1), F16); di(f"wb{s}", (128, 4, D1), F16)
        di(f"wf{s}", (128, 4, D1), F16)
        # attention biases pre-chunked [128, 4] (partition = t within chunk)
        di(f"ba{s}", (128, 4)); di(f"bb{s}", (128, 4)); di(f"bfC{s}", (128, 4))
        di(f"ba_r{s}", (1, D1), F16)
    di("lngT", (128, 4, D2)); di("lnbT", (128, 4, D2))
    di("w1s", (128, 128 * 200), F16); di("b1", (1, 200))
    di("w2a", (128, 50)); di("w2b", (72, 50)); di("b2T", (50, 1))
    di("w3T", (50, 10)); di("b3T", (10, 1))
    di("w4T", (10, 128)); di("b4T", (128, 1))
    di("p8", (128, 16), F16); di("p8t", (16, 128), F16)
    di("P16", (16, R), F16)
    di("maskbd", (128, 128), F16); di("idn", (128, 128))
    di("ones", (1, 512)); di("ones16", (1, 512), F16)
    di("ones128", (128, 1), F16)
    ap["outT"] = nc.dram_tensor("outT", [128, B], F32, kind="ExternalOutput").ap()
    return ap


def _build(nc, tc, ap, use_affine=True):
    MM = nc.tensor.matmul
    cp = nc.vector.tensor_copy
    tt = nc.vector.tensor_tensor
    act = nc.scalar.activation
    from contextlib import ExitStack
    ctx = ExitStack()
    P = ctx.enter_context(tc.tile_pool(name="persist", bufs=1))
    PX = ctx.enter_context(tc.tile_pool(name="xstage", bufs=2))
    PC = ctx.enter_context(tc.tile_pool(name="chunks", bufs=2))
    PC1 = ctx.enter_context(tc.tile_pool(name="chunks1", bufs=1))
    PW = ctx.enter_context(tc.tile_pool(name="w1stage", bufs=3))
    PP = ctx.enter_context(tc.tile_pool(name="ps", bufs=6, space="PSUM"))
    PH = ctx.enter_context(tc.tile_pool(name="ph", bufs=1, space="PSUM"))

    def ps(tag="ps"):
        return PP.tile([128, 512], F32, tag=tag, name=tag)

    # ---- persistent SBUF loads (gpsimd queue) ----
    def ld(name, shape, src, dt=None, eng=None):
        t = P.tile(list(shape), dt or F32, tag=name)
        (eng or nc.gpsimd).dma_start(t[:], src)
        return t
    w = {}
    for s in range(2):
        for nm in ("wa", "wb", "wf"):
            w[f"{nm}{s}"] = ld(f"{nm}{s}", (128, 4, D1), ap[f"{nm}{s}"], F16)
        w[f"ba{s}"] = ld(f"ba{s}", (128, 4), ap[f"ba{s}"])
        w[f"bb{s}"] = ld(f"bb{s}", (128, 4), ap[f"bb{s}"])
        w[f"bfC{s}"] = ld(f"bfC{s}", (128, 4), ap[f"bfC{s}"])
        w[f"ba_r{s}"] = ld(f"ba_r{s}", (1, D1), ap[f"ba_r{s}"], F16)
    lngT = ld("lngT", (128, 4, D2), ap["lngT"])
    lnbT = ld("lnbT", (128, 4, D2), ap["lnbT"])
    ones = ld("ones", (1, 512), ap["ones"])
    ones16 = ld("ones16", (1, 512), ap["ones16"], F16)
    ones128 = ld("ones128", (128, 1), ap["ones128"], F16)
    p8 = ld("p8", (128, 16), ap["p8"], F16)
    p8t = ld("p8t", (16, 128), ap["p8t"], F16)
    P16 = ld("P16", (16, R), ap["P16"], F16)
    maskbd = ld("maskbd", (128, 128), ap["maskbd"], F16)
    idn = ld("idn", (128, 128), ap["idn"])
    b1 = ld("b1", (1, 200), ap["b1"])
    w2a = ld("w2a", (128, 50), ap["w2a"]); w2b = ld("w2b", (72, 50), ap["w2b"])
    w3 = ld("w3", (50, 10), ap["w3T"]); w4 = ld("w4", (10, 128), ap["w4T"])
    b2T = ld("b2T", (50, 1), ap["b2T"]); b3T = ld("b3T", (10, 1), ap["b3T"])
    b4T = ld("b4T", (128, 1), ap["b4T"])

    SlP = P.tile([16, 64], F32, tag="SlP", name="SlP")
    xpT = [P.tile([128, R], F16, tag=f"xpT{c}", name=f"xpT{c}") for c in range(4)]
    nx = [P.tile([128, R], F16, tag=f"nx{c}", name=f"nx{c}") for c in range(4)]
    a_rows = [P.tile([128, D1], F16, tag=f"ar{h}", name=f"ar{h}") for h in range(NH)]
    bT2 = [P.tile([128, R], F16, tag=f"bT2_{c}", name=f"bT2_{c}") for c in range(4)]
    ctxT = [P.tile([128, R], F16, tag=f"ctxT{k}", name=f"ctxT{k}") for k in range(4)]
    E = P.tile([16, R], F16, tag="E", name="E")
    AB = P.tile([16, R], F16, tag="AB", name="AB")

    wa0, wb0 = w["wa0"], w["wb0"]
    ba0, bb0 = w["ba0"], w["bb0"]
    wb1, bb1 = w["wb1"], w["bb1"]

    # --- streamed helpers -------------------------------------------------
    def proj_chunk(dst, wm, bvC, src, rb, tag):
        """column-layout projection of rb-block rows; bias folded into the
        scalar-engine drain (Identity with per-partition bias)."""
        out = []
        for oc in range(4):
            pa = ps()
            for k in range(4):
                MM(pa[:, :512], wm[:, k, oc * 128:(oc + 1) * 128],
                   src[k][:, rb * 512:(rb + 1) * 512], start=(k == 0), stop=(k == 3))
            if dst is None:
                t = PC1.tile([128, 512], F16, tag=f"{tag}{oc}", name=f"{tag}{oc}")
                act(out=t[:], in_=pa[:, :512], func=IDENT, bias=bvC[:, oc:oc + 1])
                out.append(t)
            else:
                act(out=dst[oc][:, rb * 512:(rb + 1) * 512], in_=pa[:, :512],
                    func=IDENT, bias=bvC[:, oc:oc + 1])
        return out

    def arow_block(h, prevT, wm, bv_row):
        pa = ps()
        for k in range(4):
            MM(pa[:, :D1], prevT[k][:, h * 128:(h + 1) * 128], wm[:, k, :],
               start=(k == 0), stop=False)
        MM(pa[:, :D1], ones16[0:1, 0:128], bv_row[:], start=False, stop=True)
        act(out=a_rows[h][:], in_=pa[:, :D1], func=COPY)

    def scores_block(h, aTc, bTc, h0):
        """em/exp/mask for h-block h, then compact into E[:, h*128:...]."""
        pc = ps()
        o = (h - h0) * 128
        for k in range(4):
            MM(pc[:, :128], bTc[k][:, o:o + 128], aTc[k][:, o:o + 128],
               start=(k == 0), stop=(k == 3))
        e = PC.tile([128, 128], F16, tag="em", name="em")
        act(out=e[:], in_=pc[:, :128], func=EXP, scale=ISD)
        nc.gpsimd.tensor_tensor(e[:], e[:], maskbd[:], MUL)
        pe = ps()
        MM(pe[:16, :128], p8[:], e[:], start=True, stop=True)
        act(out=E[:, h * 128:(h + 1) * 128], in_=pe[:16, :128], func=COPY)

    # ---- stream phase: x DMA + pooling + stack-0 front + fc_b of both ----
    MAX = mybir.AluOpType.max
    for h in range(NH):
        for c in range(4):
            xt = PX.tile([128, NB, D2 * HWP], F16, tag="xt", name="xt")
            dma_eng = nc.sync if c % 2 == 0 else nc.scalar
            dma_eng.dma_start(xt[:], ap["xr"][c, :, h * NB:(h + 1) * NB, :])
            xv = xt.rearrange("p n (k h) -> p (n k) h", h=HWP)
            t1 = PC.tile([128, NB * D2, 26], F16, tag="t1", name="t1")
            tt(t1[:], xv[:, :, 0:26], xv[:, :, 26:52], MAX)
            tt(t1[:, :, 0:12], t1[:, :, 0:12], t1[:, :, 14:26], MAX)
            nc.vector.reduce_max(
                out=xpT[c][:, h * NB * D2:(h + 1) * NB * D2],
                in_=t1[:, :, 0:14], axis=X)
        arow_block(h, xpT, wa0, w["ba_r0"])
        if h % 4 == 3:
            rb = h // 4
            aTc = proj_chunk(None, wa0, ba0, xpT, rb, "aTc")
            bTc = proj_chunk(None, wb0, bb0, xpT, rb, "bTc")
            for hh in range(rb * 4, rb * 4 + 4):
                scores_block(hh, aTc, bTc, rb * 4)
            nc.vector.reduce_sum(
                out=SlP[:, rb * 16:(rb + 1) * 16],
                in_=E[:, rb * 512:(rb + 1) * 512]
                .rearrange("d (b c) -> d c b", c=16), axis=X)
            proj_chunk(bT2, wb1, bb1, xpT, rb, None)

    # prefetch classifier w1 chunks on the (now idle) sync queue
    w1c = [PW.tile([128, 8, 200], F16, tag="w1c", name="w1c") for _ in range(16)]
    for ci in range(16):
        nc.sync.dma_start(w1c[ci][:], ap["w1s"][:, ci * 1600:(ci + 1) * 1600]
                          .rearrange("p (j o) -> p j o", o=200))

    # --- post-stream per-stack tail --------------------------------------
    def stack_tail(s, prevT, out_inplace, pe_filler=None):
        """softmax -> bd -> ctx -> LN -> relu -> fc1 + residual."""
        wf, bfC = w[f"wf{s}"], w[f"bfC{s}"]
        Sl = P.tile([16, 16], F32, tag="Sl", name="Sl")
        tt(Sl[:], SlP[:, 0:16], SlP[:, 16:32], ADD)
        tt(Sl[:], Sl[:], SlP[:, 32:48], ADD)
        tt(Sl[:], Sl[:], SlP[:, 48:64], ADD)
        rS = P.tile([16, 16], F32, tag="rS", name="rS")
        nc.vector.reciprocal(out=rS[:], in_=Sl[:])
        prt = ps()
        nc.tensor.transpose(prt[:16, :16], rS[:], idn[0:16, 0:16])
        rST = P.tile([16, 16], F16, tag="rST", name="rST")
        act(out=rST[:], in_=prt[:16, :16], func=COPY)
        for rb in range(NRB):
            pab = ps()
            MM(pab[:16, :512], rST[:], P16[:, rb * 512:(rb + 1) * 512],
               start=True, stop=True)
            tt(AB[:, rb * 512:(rb + 1) * 512], E[:, rb * 512:(rb + 1) * 512],
               pab[:16, :512], MUL)
        # bd blocks + ctx + LN stats, per rb chunk
        st = P.tile([1, 2 * B + 2], F32, tag="st", name="st")
        nc.vector.memset(st[:, 2 * B:2 * B + 1], 1e-5)
        bd = []
        for h in range(NH):
            pb = ps()
            MM(pb[:, :128], p8t[:], AB[:, h * 128:(h + 1) * 128],
               start=True, stop=True)
            t = PC1.tile([128, 128], F16, tag=f"bd{h % 4}", name=f"bd{h % 4}")
            tt(t[:], pb[:, :128], maskbd[:], MUL)
            bd.append(t)
            if h % 4 == 3:
                rb = h // 4
                p1 = PH.tile([128, 512], F32, tag="p1", name="p1")
                for k in range(4):
                    pc = ps()
                    for hh in range(4):
                        MM(pc[:, hh * 128:(hh + 1) * 128],
                           a_rows[rb * 4 + hh][:, k * 128:(k + 1) * 128],
                           bd[hh][:], start=True, stop=True,
                           skip_group_check=True)
                    act(out=ctxT[k][:, rb * 512:(rb + 1) * 512], in_=pc[:, :512],
                        func=COPY)
                    sq = PC.tile([128, 512], F16, tag="sq", name="sq")
                    nc.gpsimd.tensor_tensor(sq[:], ctxT[k][:, rb * 512:(rb + 1) * 512],
                                            ctxT[k][:, rb * 512:(rb + 1) * 512], MUL)
                    MM(p1[0:1, :512], ones128[:], ctxT[k][:, rb * 512:(rb + 1) * 512],
                       start=(k == 0), stop=(k == 3), skip_group_check=True)
                    MM(p1[32:33, :512], ones128[:], sq[:],
                       start=(k == 0), stop=(k == 3), skip_group_check=True)
                    if k == 3:
                        nc.vector.reduce_sum(
                            out=st[:, rb * 32:(rb + 1) * 32],
                            in_=p1[0:1, :512].rearrange("o (b c) -> o b c", c=16),
                            axis=X)
                        nc.vector.reduce_sum(
                            out=st[:, B + rb * 32:B + (rb + 1) * 32],
                            in_=p1[32:33, :512].rearrange("o (b c) -> o b c", c=16),
                            axis=X)
                bd = []
        # batch stats: mean/rstd per batch
        nc.vector.tensor_scalar_mul(st[:, 0:B], st[:, 0:B], 1.0 / 8192.0)
        nc.vector.tensor_scalar_mul(st[:, B:2 * B], st[:, B:2 * B], 1.0 / 8192.0)
        tmp = P.tile([1, B], F32, tag="lntmp", name="lntmp")
        tt(tmp[:], st[:, 0:B], st[:, 0:B], MUL)
        tt(st[:, B:2 * B], st[:, B:2 * B], tmp[:], SUB)
        act(out=st[:, B:2 * B], in_=st[:, B:2 * B], func=SQRT,
            bias=st[:, 2 * B:2 * B + 1])
        nc.vector.reciprocal(out=st[:, B:2 * B], in_=st[:, B:2 * B])
        # row vectors [1, R] f16: mean/rstd repeated over the 16 c of each b
        mrow = P.tile([1, R], F16, tag="mrow", name="mrow")
        rrow = P.tile([1, R], F16, tag="rrow", name="rrow")
        cp(out=mrow.rearrange("o (b c) -> o b c", c=16),
           in_=st[:, 0:B, None].to_broadcast((1, B, 16)))
        cp(out=rrow.rearrange("o (b c) -> o b c", c=16),
           in_=st[:, B:2 * B, None].to_broadcast((1, B, 16)))
        # expand to [128, R] via rank-1 matmuls, drain f16 into the xt ring
        mE = PX.tile([128, NB, D2 * HWP], F16, tag="xt", name="xt_m")
        rE = PX.tile([128, NB, D2 * HWP], F16, tag="xt", name="xt_r")
        mEf = mE.rearrange("p n h -> p (n h)")
        rEf = rE.rearrange("p n h -> p (n h)")
        for rb in range(NRB):
            pm = ps()
            MM(pm[:, :512], ones16[0:1, 0:128], mrow[:, rb * 512:(rb + 1) * 512],
               start=True, stop=True)
            act(out=mEf[:, rb * 512:(rb + 1) * 512], in_=pm[:, :512], func=COPY)
            pr = ps()
            MM(pr[:, :512], ones16[0:1, 0:128], rrow[:, rb * 512:(rb + 1) * 512],
               start=True, stop=True)
            act(out=rEf[:, rb * 512:(rb + 1) * 512], in_=pr[:, :512], func=COPY)
        # normalize in place (flat contiguous f16 -> DVE 2x), then relu
        for k in range(4):
            tt(ctxT[k][:], ctxT[k][:], mEf[:, :R], SUB)
            tt(ctxT[k][:], ctxT[k][:], rEf[:, :R], MUL)
            if use_affine:
                vcb = ctxT[k].rearrange("p (b c) -> p c b", c=16)
                tt(vcb, vcb, lngT[:, k, :, None].to_broadcast((128, 16, B)), MUL)
                tt(vcb, vcb, lnbT[:, k, :, None].to_broadcast((128, 16, B)), ADD)
            act(out=ctxT[k][:], in_=ctxT[k][:], func=RELU)
        if pe_filler is not None:
            pe_filler()
        # fc1 + residual (bias folded into the scalar drain; f16 adds)
        for rb in range(NRB):
            for oc in range(4):
                pn = ps()
                for k in range(4):
                    MM(pn[:, :512], wf[:, k, oc * 128:(oc + 1) * 128],
                       ctxT[k][:, rb * 512:(rb + 1) * 512], start=(k == 0),
                       stop=(k == 3))
                f1 = PC.tile([128, 512], F16, tag="f1", name="f1")
                act(out=f1[:], in_=pn[:, :512], func=IDENT, bias=bfC[:, oc:oc + 1])
                if out_inplace:
                    tt(nx[oc][:, rb * 512:(rb + 1) * 512], f1[:],
                       nx[oc][:, rb * 512:(rb + 1) * 512], ADD)
                else:
                    tt(nx[oc][:, rb * 512:(rb + 1) * 512], f1[:],
                       prevT[oc][:, rb * 512:(rb + 1) * 512], ADD)

    ph = PH.tile([128, 512], F32, tag="ph", name="ph")

    def _cls_xp(ci0, ci1):
        # classifier xp-half piece: fills a PE gap while LN output is pending
        for ci in range(ci0, ci1):
            for jj in range(8):
                j = ci * 8 + jj
                d2 = j // 4
                c = j % 4
                lhsT = xpT[c].rearrange("p (b c16) -> p c16 b", c16=16)[:, d2, :]
                MM(ph[:, :200], lhsT, w1c[ci][:, jj, :],
                   start=(j == 0), stop=False, skip_group_check=True)

    def cls_xp_half1():
        _cls_xp(0, 4)

    def cls_xp_half2():
        _cls_xp(4, 8)

    stack_tail(0, xpT, out_inplace=False, pe_filler=cls_xp_half1)

    # --- stack 1: fc_a/scores from nx, fc_b(xp) already in bT2 -----------
    wa1, ba1 = w["wa1"], w["ba1"]
    for rb in range(NRB):
        for h in range(rb * 4, rb * 4 + 4):
            arow_block(h, nx, wa1, w["ba_r1"])
        aTc = proj_chunk(None, wa1, ba1, nx, rb, "aTc")
        for h in range(rb * 4, rb * 4 + 4):
            pc = ps()
            for k in range(4):
                MM(pc[:, :128], bT2[k][:, h * 128:(h + 1) * 128],
                   aTc[k][:, (h - rb * 4) * 128:(h - rb * 4 + 1) * 128],
                   start=(k == 0), stop=(k == 3))
            e = PC.tile([128, 128], F16, tag="em", name="em")
            act(out=e[:], in_=pc[:, :128], func=EXP, scale=ISD)
            nc.gpsimd.tensor_tensor(e[:], e[:], maskbd[:], MUL)
            pe = ps()
            MM(pe[:16, :128], p8[:], e[:], start=True, stop=True)
            act(out=E[:, h * 128:(h + 1) * 128], in_=pe[:16, :128], func=COPY)
        nc.vector.reduce_sum(
            out=SlP[:, rb * 16:(rb + 1) * 16],
            in_=E[:, rb * 512:(rb + 1) * 512]
            .rearrange("d (b c) -> d c b", c=16), axis=X)
    stack_tail(1, nx, out_inplace=True, pe_filler=cls_xp_half2)

    # ---- classifier: nx half ----
    j = 64
    for ci in range(8, 16):
        for jj in range(8):
            d2 = (j % 64) // 4
            c = j % 4
            lhsT = nx[c].rearrange("p (b c16) -> p c16 b", c16=16)[:, d2, :]
            MM(ph[:, :200], lhsT, w1c[ci][:, jj, :],
               start=False, stop=False, skip_group_check=True)
            j += 1
    MM(ph[:, :200], ones[0:1, 0:128], b1[:], start=False, stop=True,
       skip_group_check=True)
    h1 = P.tile([128, 200], F32, tag="h1", name="h1")
    act(out=h1[:], in_=ph[:, :200], func=RELU)
    # transpose h1 -> [200, 128] in two pieces
    pt = ps()
    nc.tensor.transpose(pt[:, :128], h1[:, 0:128], idn[:])
    h1a = P.tile([128, 128], F32, tag="h1a", name="h1a"); cp(out=h1a[:], in_=pt[:, :128])
    pt2 = ps()
    nc.tensor.transpose(pt2[:72, :128], h1[:, 128:200], idn[:])
    h1b = P.tile([72, 128], F32, tag="h1b", name="h1b"); cp(out=h1b[:], in_=pt2[:72, :128])
    p2 = ps()
    MM(p2[:50, :B], w2a[:], h1a[:], start=True, stop=False)
    MM(p2[:50, :B], w2b[:], h1b[:], start=False, stop=True)
    h2 = P.tile([50, B], F32, tag="h2", name="h2")
    act(out=h2[:], in_=p2[:50, :B], func=RELU, bias=b2T[:])
    p3 = ps()
    MM(p3[:10, :B], w3[:], h2[:], start=True, stop=True)
    h3 = P.tile([10, B], F32, tag="h3", name="h3")
    act(out=h3[:], in_=p3[:10, :B], func=RELU, bias=b3T[:])
    p4 = ps()
    MM(p4[:, :B], w4[:], h3[:], start=True, stop=True)
    oT = P.tile([128, B], F32, tag="oT", name="oT")
    tt(oT[:], p4[:, :B], b4T[:, 0, None].to_broadcast((128, B)), ADD)
    nc.sync.dma_start(ap["outT"], oT[:])
    ctx.close()


def build_program(use_affine=None):
    if use_affine is None:
        use_affine = USE_AFFINE[0]
    nc = bacc.Bacc("TRN2", target_bir_lowering=False, debug=False, num_devices=1)
    ap = _declare(nc)
    with tile.TileContext(nc) as tc:
        _build(nc, tc, ap, use_affine=use_affine)
    nc.compile()
    return nc


def make_in_maps(inputs):
    x = np.asarray(inputs["x"], np.float32)
    aw = np.asarray(inputs["attn_w"], np.float32)
    ab = np.asarray(inputs["attn_b"], np.float32)
    ln_g = np.asarray(inputs["ln_g"], np.float32)
    ln_b = np.asarray(inputs["ln_b"], np.float32)
    USE_AFFINE[0] = not (np.all(ln_g == 1.0) and np.all(ln_b == 0.0))
    I16 = np.eye(16, dtype=np.float32)

    def chunkT(mat):  # [512, out...] -> [128, 4, out] contiguous
        return np.ascontiguousarray(mat.reshape(4, 128, -1).transpose(1, 0, 2))

    # x: [B, D1, D2, 7, 7] -> f16 [4, 128, B, D2*HWP] with hw padded 49->50
    x16 = x.astype(np.float16).reshape(B, 4, 128, D2, HW)
    xp_pad = np.concatenate([x16] + [x16[..., -1:]] * (HWP - HW), axis=-1)
    xr = np.ascontiguousarray(
        xp_pad.reshape(B, 4, 128, D2 * HWP).transpose(1, 2, 0, 3))

    m = {
        "xr": xr,
        "lngT": chunkT(np.ascontiguousarray(ln_g.T)),
        "lnbT": chunkT(np.ascontiguousarray(ln_b.T)),
        "w1s": np.ascontiguousarray(
            np.asarray(inputs["w1"], np.float32).T.reshape(128, 128, 200)
            .transpose(1, 0, 2)).reshape(128, 128 * 200).astype(np.float16),
        "b1": np.asarray(inputs["b1"], np.float32).reshape(1, 200),
        "w2a": np.ascontiguousarray(np.asarray(inputs["w2"], np.float32).T[0:128]),
        "w2b": np.ascontiguousarray(np.asarray(inputs["w2"], np.float32).T[128:200]),
        "b2T": np.asarray(inputs["b2"], np.float32).reshape(50, 1),
        "w3T": np.ascontiguousarray(np.asarray(inputs["w3"], np.float32).T),
        "b3T": np.asarray(inputs["b3"], np.float32).reshape(10, 1),
        "w4T": np.ascontiguousarray(np.asarray(inputs["w4"], np.float32).T),
        "b4T": np.asarray(inputs["b4"], np.float32).reshape(128, 1),
        "p8": np.tile(I16, (8, 1)).astype(np.float16),
        "P16": np.tile(I16, (1, 128)).astype(np.float16),
        "p8t": np.tile(I16, (1, 8)).astype(np.float16),
        "maskbd": np.kron(np.eye(8, dtype=np.float32),
                          np.ones((16, 16), np.float32)).astype(np.float16),
        "idn": np.eye(128, dtype=np.float32),
        "ones": np.ones((1, 512), np.float32),
        "ones16": np.ones((1, 512), np.float16),
        "ones128": np.ones((128, 1), np.float16),
    }
    for s in range(2):
        m[f"wa{s}"] = chunkT(np.ascontiguousarray(aw[s, 0].T)).astype(np.float16)
        m[f"wb{s}"] = chunkT(np.ascontiguousarray(aw[s, 1].T)).astype(np.float16)
        m[f"wf{s}"] = chunkT(np.ascontiguousarray(aw[s, 3].T)).astype(np.float16)
        m[f"ba{s}"] = np.ascontiguousarray(ab[s, 0].reshape(4, 128).T)
        m[f"bb{s}"] = np.ascontiguousarray(ab[s, 1].reshape(4, 128).T)
        m[f"bfC{s}"] = np.ascontiguousarray(ab[s, 3].reshape(4, 128).T)
        m[f"ba_r{s}"] = ab[s, 0].reshape(1, D1).astype(np.float16)
    return [m]


def kernel(**inputs):
    in_maps = make_in_maps(inputs)
    nc = build_program()
    res = run_bass_kernel_spmd(
        nc, in_maps, core_ids=[0],
        trace=bool(os.environ.get("KTRACE")))
    LAST["results"] = res
    return res.results[0]["outT"].T.astype(np.float32)


# revision 4
# speedup vs baseline: 1.0215x; 1.0215x over previous
"""Trainium2 Bass kernel for nn_LongTermFeatureBank — single-core version.

Rationale: the module's legacy softmax over dim=0 (batch) makes any
batch-sharded multi-core layout depend on a collective; in the grading
environment the collective entry stalls on the slowest-to-start core,
which dominated the old 8-core kernel's measured time.  This version
runs the full batch on one core with fp16 inputs/weights (f32
accumulation), so the measured span is pure local DMA + compute.

Performance notes:
  * x rows are host-padded from 49 to 50 fp16 elements so every DVE
    access is 4B-aligned — required for the vector engine's 2x packed
    mode on the pooling reduce.
  * PSUM->SBUF drains ride the scalar engine (activation Copy /
    Identity-with-bias), keeping the vector engine free for pooling.
  * LayerNorm mean/rstd are expanded to full [128, R] operands with
    rank-1 matmuls so the normalize runs as flat, contiguous, all-fp16
    tensor_tensor ops (2x mode) instead of strided broadcasts.
"""
import os
import numpy as np
import ml_dtypes

import concourse.bass as bass
import concourse.tile as tile
from concourse import bacc, mybir
from concourse.bass_utils import run_bass_kernel_spmd

F32 = mybir.dt.float32
F16 = mybir.dt.float16
X = mybir.AxisListType.X
ADD = mybir.AluOpType.add
SUB = mybir.AluOpType.subtract
MUL = mybir.AluOpType.mult
EXP = mybir.ActivationFunctionType.Exp
RELU = mybir.ActivationFunctionType.Relu
SQRT = mybir.ActivationFunctionType.Sqrt
COPY = mybir.ActivationFunctionType.Copy
IDENT = mybir.ActivationFunctionType.Identity

B, D1, D2, HW = 128, 512, 16, 49
HWP = 50              # host-padded pooling window (4B alignment for DVE 2x)
NC = 1
R = B * D2            # 2048 rows (b,d2)
NH = R // 128         # 16 h-blocks
NRB = R // 512        # 4 rb-blocks
ISD = float(1.0 / np.sqrt(D1))
NB = 8                # batches per x DMA chunk -> one h-block per chunk
LAST = {}
USE_AFFINE = [True]   # set by make_in_maps from actual ln_g/ln_b values


def _declare(nc):
    ap = {}
    def di(name, shape, dt=F32):
        ap[name] = nc.dram_tensor(name, list(shape), dt, kind="ExternalInput").ap()
    di("xr", (4, 128, B, D2 * HWP), F16)
    for s in range(2):
        di(f"wa{s}", (128, 4, D1), F16); di(f"wb{s}", (128, 4, D1), F16)
        di(f"wf{s}", (128, 4, D1), F16)
        # attention biases pre-chunked [128, 4] (partition = t within chunk)
        di(f"ba{s}", (128, 4)); di(f"bb{s}", (128, 4)); di(f"bfC{s}", (128, 4))
        di(f"ba_r{s}", (1, D1), F16)
    di("lngT", (128, 4, D2)); di("lnbT", (128, 4, D2))
    di("w1s", (128, 128 * 200), F16); di("b1", (1, 200))
    di("w2a", (128, 50)); di("w2b", (72, 50)); di("b2T", (50, 1))
    di("w3T", (50, 10)); di("b3T", (10, 1))
    di("w4T", (10, 128)); di("b4T", (128, 1))
    di("p8", (128, 16), F16); di("p8t", (16, 128), F16)
    di("P16", (16, R), F16)
    di("maskbd", (128, 128), F16); di("idn", (128, 128))
    di("ones", (1, 512)); di("ones16", (1, 512), F16)
    di("ones128", (128, 1), F16)
    ap["outT"] = nc.dram_tensor("outT", [128, B], F32, kind="ExternalOutput").ap()
    return ap


def _build(nc, tc, ap, use_affine=True):
    MM = nc.tensor.matmul
    cp = nc.vector.tensor_copy
    tt = nc.vector.tensor_tensor
    act = nc.scalar.activation
    from contextlib import ExitStack
    ctx = ExitStack()
    P = ctx.enter_context(tc.tile_pool(name="persist", bufs=1))
    PX = ctx.enter_context(tc.tile_pool(name="xstage", bufs=2))
    PC = ctx.enter_context(tc.tile_pool(name="chunks", bufs=2))
    PC1 = ctx.enter_context(tc.tile_pool(name="chunks1", bufs=1))
    PW = ctx.enter_context(tc.tile_pool(name="w1stage", bufs=3))
    PP = ctx.enter_context(tc.tile_pool(name="ps", bufs=6, space="PSUM"))
    PH = ctx.enter_context(tc.tile_pool(name="ph", bufs=1, space="PSUM"))

    def ps(tag="ps"):
        return PP.tile([128, 512], F32, tag=tag, name=tag)

    # ---- persistent SBUF loads (gpsimd queue) ----
    def ld(name, shape, src, dt=None, eng=None):
        t = P.tile(list(shape), dt or F32, tag=name)
        (eng or nc.gpsimd).dma_start(t[:], src)
        return t
    w = {}
    for s in range(2):
        for nm in ("wa", "wb", "wf"):
            w[f"{nm}{s}"] = ld(f"{nm}{s}", (128, 4, D1), ap[f"{nm}{s}"], F16)
        w[f"ba{s}"] = ld(f"ba{s}", (128, 4), ap[f"ba{s}"])
        w[f"bb{s}"] = ld(f"bb{s}", (128, 4), ap[f"bb{s}"])
        w[f"bfC{s}"] = ld(f"bfC{s}", (128, 4), ap[f"bfC{s}"])
        w[f"ba_r{s}"] = ld(f"ba_r{s}", (1, D1), ap[f"ba_r{s}"], F16)
    lngT = ld("lngT", (128, 4, D2), ap["lngT"])
    lnbT = ld("lnbT", (128, 4, D2), ap["lnbT"])
    ones = ld("ones", (1, 512), ap["ones"])
    ones16 = ld("ones16", (1, 512), ap["ones16"], F16)
    ones128 = ld("ones128", (128, 1), ap["ones128"], F16)
    p8 = ld("p8", (128, 16), ap["p8"], F16)
    p8t = ld("p8t", (16, 128), ap["p8t"], F16)
    P16 = ld("P16", (16, R), ap["P16"], F16)
    maskbd = ld("maskbd", (128, 128), ap["maskbd"], F16)
    idn = ld("idn", (128, 128), ap["idn"])
    b1 = ld("b1", (1, 200), ap["b1"])
    w2a = ld("w2a", (128, 50), ap["w2a"]); w2b = ld("w2b", (72, 50), ap["w2b"])
    w3 = ld("w3", (50, 10), ap["w3T"]); w4 = ld("w4", (10, 128), ap["w4T"])
    b2T = ld("b2T", (50, 1), ap["b2T"]); b3T = ld("b3T", (10, 1), ap["b3T"])
    b4T = ld("b4T", (128, 1), ap["b4T"])

    SlP = P.tile([16, 64], F32, tag="SlP", name="SlP")
    xpT = [P.tile([128, R], F16, tag=f"xpT{c}", name=f"xpT{c}") for c in range(4)]
    nx = [P.tile([128, R], F16, tag=f"nx{c}", name=f"nx{c}") for c in range(4)]
    a_rows = [P.tile([128, D1], F16, tag=f"ar{h}", name=f"ar{h}") for h in range(NH)]
    bT2 = [P.tile([128, R], F16, tag=f"bT2_{c}", name=f"bT2_{c}") for c in range(4)]
    ctxT = [P.tile([128, R], F16, tag=f"ctxT{k}", name=f"ctxT{k}") for k in range(4)]
    E = P.tile([16, R], F16, tag="E", name="E")
    AB = P.tile([16, R], F16, tag="AB", name="AB")

    wa0, wb0 = w["wa0"], w["wb0"]
    ba0, bb0 = w["ba0"], w["bb0"]
    wb1, bb1 = w["wb1"], w["bb1"]

    # --- streamed helpers -------------------------------------------------
    def proj_chunk(dst, wm, bvC, src, rb, tag):
        """column-layout projection of rb-block rows; bias folded into the
        scalar-engine drain (Identity with per-partition bias)."""
        out = []
        for oc in range(4):
            pa = ps()
            for k in range(4):
                MM(pa[:, :512], wm[:, k, oc * 128:(oc + 1) * 128],
                   src[k][:, rb * 512:(rb + 1) * 512], start=(k == 0), stop=(k == 3))
            if dst is None:
                t = PC1.tile([128, 512], F16, tag=f"{tag}{oc}", name=f"{tag}{oc}")
                act(out=t[:], in_=pa[:, :512], func=IDENT, bias=bvC[:, oc:oc + 1])
                out.append(t)
            else:
                act(out=dst[oc][:, rb * 512:(rb + 1) * 512], in_=pa[:, :512],
                    func=IDENT, bias=bvC[:, oc:oc + 1])
        return out

    def arow_block(h, prevT, wm, bv_row):
        pa = ps()
        for k in range(4):
            MM(pa[:, :D1], prevT[k][:, h * 128:(h + 1) * 128], wm[:, k, :],
               start=(k == 0), stop=False)
        MM(pa[:, :D1], ones16[0:1, 0:128], bv_row[:], start=False, stop=True)
        act(out=a_rows[h][:], in_=pa[:, :D1], func=COPY)

    def scores_block(h, aTc, bTc, h0):
        """em/exp/mask for h-block h, then compact into E[:, h*128:...]."""
        pc = ps()
        o = (h - h0) * 128
        for k in range(4):
            MM(pc[:, :128], bTc[k][:, o:o + 128], aTc[k][:, o:o + 128],
               start=(k == 0), stop=(k == 3))
        e = PC.tile([128, 128], F16, tag="em", name="em")
        act(out=e[:], in_=pc[:, :128], func=EXP, scale=ISD)
        nc.gpsimd.tensor_tensor(e[:], e[:], maskbd[:], MUL)
        pe = ps()
        MM(pe[:16, :128], p8[:], e[:], start=True, stop=True)
        act(out=E[:, h * 128:(h + 1) * 128], in_=pe[:16, :128], func=COPY)

    # ---- stream phase: x DMA + pooling + stack-0 front + fc_b of both ----
    MAX = mybir.AluOpType.max
    for h in range(NH):
        for c in range(4):
            xt = PX.tile([128, NB, D2 * HWP], F16, tag="xt", name="xt")
            dma_eng = (nc.sync, nc.scalar, nc.sync, nc.gpsimd)[c]
            dma_eng.dma_start(xt[:], ap["xr"][c, :, h * NB:(h + 1) * NB, :])
            xv = xt.rearrange("p n (k h) -> p (n k) h", h=HWP)
            t1 = PC.tile([128, NB * D2, 26], F16, tag="t1", name="t1")
            # overlapping aligned split: max is idempotent, 24:50 covers the rest
            tt(t1[:], xv[:, :, 0:26], xv[:, :, 24:50], MAX)
            tt(t1[:, :, 0:12], t1[:, :, 0:12], t1[:, :, 14:26], MAX)
            nc.vector.reduce_max(
                out=xpT[c][:, h * NB * D2:(h + 1) * NB * D2],
                in_=t1[:, :, 0:14], axis=X)
        arow_block(h, xpT, wa0, w["ba_r0"])
        if h % 4 == 3:
            rb = h // 4
            aTc = proj_chunk(None, wa0, ba0, xpT, rb, "aTc")
            bTc = proj_chunk(None, wb0, bb0, xpT, rb, "bTc")
            for hh in range(rb * 4, rb * 4 + 4):
                scores_block(hh, aTc, bTc, rb * 4)
            nc.vector.reduce_sum(
                out=SlP[:, rb * 16:(rb + 1) * 16],
                in_=E[:, rb * 512:(rb + 1) * 512]
                .rearrange("d (b c) -> d c b", c=16), axis=X)
            proj_chunk(bT2, wb1, bb1, xpT, rb, None)

    # prefetch classifier w1 chunks on the (now idle) sync queue
    w1c = [PW.tile([128, 8, 200], F16, tag="w1c", name="w1c") for _ in range(16)]
    for ci in range(16):
        nc.sync.dma_start(w1c[ci][:], ap["w1s"][:, ci * 1600:(ci + 1) * 1600]
                          .rearrange("p (j o) -> p j o", o=200))

    # --- post-stream per-stack tail --------------------------------------
    def stack_tail(s, prevT, out_inplace, pe_filler=None):
        """softmax -> bd -> ctx -> LN -> relu -> fc1 + residual."""
        wf, bfC = w[f"wf{s}"], w[f"bfC{s}"]
        Sl = P.tile([16, 16], F32, tag="Sl", name="Sl")
        tt(Sl[:], SlP[:, 0:16], SlP[:, 16:32], ADD)
        tt(Sl[:], Sl[:], SlP[:, 32:48], ADD)
        tt(Sl[:], Sl[:], SlP[:, 48:64], ADD)
        rS = P.tile([16, 16], F32, tag="rS", name="rS")
        nc.vector.reciprocal(out=rS[:], in_=Sl[:])
        prt = ps()
        nc.tensor.transpose(prt[:16, :16], rS[:], idn[0:16, 0:16])
        rST = P.tile([16, 16], F16, tag="rST", name="rST")
        act(out=rST[:], in_=prt[:16, :16], func=COPY)
        for rb in range(NRB):
            pab = ps()
            MM(pab[:16, :512], rST[:], P16[:, rb * 512:(rb + 1) * 512],
               start=True, stop=True)
            tt(AB[:, rb * 512:(rb + 1) * 512], E[:, rb * 512:(rb + 1) * 512],
               pab[:16, :512], MUL)
        # bd blocks + ctx + LN stats, per rb chunk
        st = P.tile([1, 2 * B + 2], F32, tag="st", name="st")
        nc.vector.memset(st[:, 2 * B:2 * B + 1], 1e-5)
        bd = []
        for h in range(NH):
            pb = ps()
            MM(pb[:, :128], p8t[:], AB[:, h * 128:(h + 1) * 128],
               start=True, stop=True)
            t = PC1.tile([128, 128], F16, tag=f"bd{h % 4}", name=f"bd{h % 4}")
            tt(t[:], pb[:, :128], maskbd[:], MUL)
            bd.append(t)
            if h % 4 == 3:
                rb = h // 4
                p1 = PH.tile([128, 512], F32, tag="p1", name="p1")
                for k in range(4):
                    pc = ps()
                    for hh in range(4):
                        MM(pc[:, hh * 128:(hh + 1) * 128],
                           a_rows[rb * 4 + hh][:, k * 128:(k + 1) * 128],
                           bd[hh][:], start=True, stop=True,
                           skip_group_check=True)
                    act(out=ctxT[k][:, rb * 512:(rb + 1) * 512], in_=pc[:, :512],
                        func=COPY)
                    sq = PC.tile([128, 512], F16, tag="sq", name="sq")
                    nc.gpsimd.tensor_tensor(sq[:], ctxT[k][:, rb * 512:(rb + 1) * 512],
                                            ctxT[k][:, rb * 512:(rb + 1) * 512], MUL)
                    MM(p1[0:1, :512], ones128[:], ctxT[k][:, rb * 512:(rb + 1) * 512],
                       start=(k == 0), stop=(k == 3), skip_group_check=True)
                    MM(p1[32:33, :512], ones128[:], sq[:],
                       start=(k == 0), stop=(k == 3), skip_group_check=True)
                    if k == 3:
                        nc.vector.reduce_sum(
                            out=st[:, rb * 32:(rb + 1) * 32],
                            in_=p1[0:1, :512].rearrange("o (b c) -> o b c", c=16),
                            axis=X)
                        nc.vector.reduce_sum(
                            out=st[:, B + rb * 32:B + (rb + 1) * 32],
                            in_=p1[32:33, :512].rearrange("o (b c) -> o b c", c=16),
                            axis=X)
                bd = []
        # batch stats: mean/rstd per batch
        nc.vector.tensor_scalar_mul(st[:, 0:B], st[:, 0:B], 1.0 / 8192.0)
        nc.vector.tensor_scalar_mul(st[:, B:2 * B], st[:, B:2 * B], 1.0 / 8192.0)
        tmp = P.tile([1, B], F32, tag="lntmp", name="lntmp")
        tt(tmp[:], st[:, 0:B], st[:, 0:B], MUL)
        tt(st[:, B:2 * B], st[:, B:2 * B], tmp[:], SUB)
        act(out=st[:, B:2 * B], in_=st[:, B:2 * B], func=SQRT,
            bias=st[:, 2 * B:2 * B + 1])
        nc.vector.reciprocal(out=st[:, B:2 * B], in_=st[:, B:2 * B])
        # row vectors [1, R] f16: mean/rstd repeated over the 16 c of each b
        mrow = P.tile([1, R], F16, tag="mrow", name="mrow")
        rrow = P.tile([1, R], F16, tag="rrow", name="rrow")
        cp(out=mrow.rearrange("o (b c) -> o b c", c=16),
           in_=st[:, 0:B, None].to_broadcast((1, B, 16)))
        cp(out=rrow.rearrange("o (b c) -> o b c", c=16),
           in_=st[:, B:2 * B, None].to_broadcast((1, B, 16)))
        # expand to [128, R] via rank-1 matmuls, drain f16 into the xt ring
        mE = PX.tile([128, NB, D2 * HWP], F16, tag="xt", name="xt_m")
        rE = PX.tile([128, NB, D2 * HWP], F16, tag="xt", name="xt_r")
        mEf = mE.rearrange("p n h -> p (n h)")
        rEf = rE.rearrange("p n h -> p (n h)")
        for rb in range(NRB):
            pm = ps()
            MM(pm[:, :512], ones16[0:1, 0:128], mrow[:, rb * 512:(rb + 1) * 512],
               start=True, stop=True)
            act(out=mEf[:, rb * 512:(rb + 1) * 512], in_=pm[:, :512], func=COPY)
            pr = ps()
            MM(pr[:, :512], ones16[0:1, 0:128], rrow[:, rb * 512:(rb + 1) * 512],
               start=True, stop=True)
            act(out=rEf[:, rb * 512:(rb + 1) * 512], in_=pr[:, :512], func=COPY)
        # normalize in place (flat contiguous f16 -> DVE 2x), then relu
        for k in range(4):
            tt(ctxT[k][:], ctxT[k][:], mEf[:, :R], SUB)
            tt(ctxT[k][:], ctxT[k][:], rEf[:, :R], MUL)
            if use_affine:
                vcb = ctxT[k].rearrange("p (b c) -> p c b", c=16)
                tt(vcb, vcb, lngT[:, k, :, None].to_broadcast((128, 16, B)), MUL)
                tt(vcb, vcb, lnbT[:, k, :, None].to_broadcast((128, 16, B)), ADD)
            act(out=ctxT[k][:], in_=ctxT[k][:], func=RELU)
        if pe_filler is not None:
            pe_filler()
        # fc1 + residual (bias folded into the scalar drain; f16 adds)
        for rb in range(NRB):
            for oc in range(4):
                pn = ps()
                for k in range(4):
                    MM(pn[:, :512], wf[:, k, oc * 128:(oc + 1) * 128],
                       ctxT[k][:, rb * 512:(rb + 1) * 512], start=(k == 0),
                       stop=(k == 3))
                f1 = PC.tile([128, 512], F16, tag="f1", name="f1")
                act(out=f1[:], in_=pn[:, :512], func=IDENT, bias=bfC[:, oc:oc + 1])
                if out_inplace:
                    tt(nx[oc][:, rb * 512:(rb + 1) * 512], f1[:],
                       nx[oc][:, rb * 512:(rb + 1) * 512], ADD)
                else:
                    tt(nx[oc][:, rb * 512:(rb + 1) * 512], f1[:],
                       prevT[oc][:, rb * 512:(rb + 1) * 512], ADD)

    ph = PH.tile([128, 512], F32, tag="ph", name="ph")

    def _cls_xp(ci0, ci1):
        # classifier xp-half piece: fills a PE gap while LN output is pending
        for ci in range(ci0, ci1):
            for jj in range(8):
                j = ci * 8 + jj
                d2 = j // 4
                c = j % 4
                lhsT = xpT[c].rearrange("p (b c16) -> p c16 b", c16=16)[:, d2, :]
                MM(ph[:, :200], lhsT, w1c[ci][:, jj, :],
                   start=(j == 0), stop=False, skip_group_check=True)

    def cls_xp_half1():
        _cls_xp(0, 4)

    def cls_xp_half2():
        _cls_xp(4, 8)

    stack_tail(0, xpT, out_inplace=False, pe_filler=cls_xp_half1)

    # --- stack 1: fc_a/scores from nx, fc_b(xp) already in bT2 -----------
    wa1, ba1 = w["wa1"], w["ba1"]
    for rb in range(NRB):
        for h in range(rb * 4, rb * 4 + 4):
            arow_block(h, nx, wa1, w["ba_r1"])
        aTc = proj_chunk(None, wa1, ba1, nx, rb, "aTc")
        for h in range(rb * 4, rb * 4 + 4):
            pc = ps()
            for k in range(4):
                MM(pc[:, :128], bT2[k][:, h * 128:(h + 1) * 128],
                   aTc[k][:, (h - rb * 4) * 128:(h - rb * 4 + 1) * 128],
                   start=(k == 0), stop=(k == 3))
            e = PC.tile([128, 128], F16, tag="em", name="em")
            act(out=e[:], in_=pc[:, :128], func=EXP, scale=ISD)
            nc.gpsimd.tensor_tensor(e[:], e[:], maskbd[:], MUL)
            pe = ps()
            MM(pe[:16, :128], p8[:], e[:], start=True, stop=True)
            act(out=E[:, h * 128:(h + 1) * 128], in_=pe[:16, :128], func=COPY)
        nc.vector.reduce_sum(
            out=SlP[:, rb * 16:(rb + 1) * 16],
            in_=E[:, rb * 512:(rb + 1) * 512]
            .rearrange("d (b c) -> d c b", c=16), axis=X)
    stack_tail(1, nx, out_inplace=True, pe_filler=cls_xp_half2)

    # ---- classifier: nx half ----
    j = 64
    for ci in range(8, 16):
        for jj in range(8):
            d2 = (j % 64) // 4
            c = j % 4
            lhsT = nx[c].rearrange("p (b c16) -> p c16 b", c16=16)[:, d2, :]
            MM(ph[:, :200], lhsT, w1c[ci][:, jj, :],
               start=False, stop=False, skip_group_check=True)
            j += 1
    MM(ph[:, :200], ones[0:1, 0:128], b1[:], start=False, stop=True,
       skip_group_check=True)
    h1 = P.tile([128, 200], F32, tag="h1", name="h1")
    act(out=h1[:], in_=ph[:, :200], func=RELU)
    # transpose h1 -> [200, 128] in two pieces
    pt = ps()
    nc.tensor.transpose(pt[:, :128], h1[:, 0:128], idn[:])
    h1a = P.tile([128, 128], F32, tag="h1a", name="h1a"); cp(out=h1a[:], in_=pt[:, :128])
    pt2 = ps()
    nc.tensor.transpose(pt2[:72, :128], h1[:, 128:200], idn[:])
    h1b = P.tile([72, 128], F32, tag="h1b", name="h1b"); cp(out=h1b[:], in_=pt2[:72, :128])
    p2 = ps()
    MM(p2[:50, :B], w2a[:], h1a[:], start=True, stop=False)
    MM(p2[:50, :B], w2b[:], h1b[:], start=False, stop=True)
    h2 = P.tile([50, B], F32, tag="h2", name="h2")
    act(out=h2[:], in_=p2[:50, :B], func=RELU, bias=b2T[:])
    p3 = ps()
    MM(p3[:10, :B], w3[:], h2[:], start=True, stop=True)
    h3 = P.tile([10, B], F32, tag="h3", name="h3")
    act(out=h3[:], in_=p3[:10, :B], func=RELU, bias=b3T[:])
    p4 = ps()
    MM(p4[:, :B], w4[:], h3[:], start=True, stop=True)
    oT = P.tile([128, B], F32, tag="oT", name="oT")
    tt(oT[:], p4[:, :B], b4T[:, 0, None].to_broadcast((128, B)), ADD)
    nc.sync.dma_start(ap["outT"], oT[:])
    ctx.close()


def build_program(use_affine=None):
    if use_affine is None:
        use_affine = USE_AFFINE[0]
    nc = bacc.Bacc("TRN2", target_bir_lowering=False, debug=False, num_devices=1)
    ap = _declare(nc)
    with tile.TileContext(nc) as tc:
        _build(nc, tc, ap, use_affine=use_affine)
    nc.compile()
    return nc


def make_in_maps(inputs):
    x = np.asarray(inputs["x"], np.float32)
    aw = np.asarray(inputs["attn_w"], np.float32)
    ab = np.asarray(inputs["attn_b"], np.float32)
    ln_g = np.asarray(inputs["ln_g"], np.float32)
    ln_b = np.asarray(inputs["ln_b"], np.float32)
    USE_AFFINE[0] = not (np.all(ln_g == 1.0) and np.all(ln_b == 0.0))
    I16 = np.eye(16, dtype=np.float32)

    def chunkT(mat):  # [512, out...] -> [128, 4, out] contiguous
        return np.ascontiguousarray(mat.reshape(4, 128, -1).transpose(1, 0, 2))

    # x: [B, D1, D2, 7, 7] -> f16 [4, 128, B, D2*HWP] with hw padded 49->50
    x16 = x.astype(np.float16).reshape(B, 4, 128, D2, HW)
    xp_pad = np.concatenate([x16] + [x16[..., -1:]] * (HWP - HW), axis=-1)
    xr = np.ascontiguousarray(
        xp_pad.reshape(B, 4, 128, D2 * HWP).transpose(1, 2, 0, 3))

    m = {
        "xr": xr,
        "lngT": chunkT(np.ascontiguousarray(ln_g.T)),
        "lnbT": chunkT(np.ascontiguousarray(ln_b.T)),
        "w1s": np.ascontiguousarray(
            np.asarray(inputs["w1"], np.float32).T.reshape(128, 128, 200)
            .transpose(1, 0, 2)).reshape(128, 128 * 200).astype(np.float16),
        "b1": np.asarray(inputs["b1"], np.float32).reshape(1, 200),
        "w2a": np.ascontiguousarray(np.asarray(inputs["w2"], np.float32).T[0:128]),
        "w2b": np.ascontiguousarray(np.asarray(inputs["w2"], np.float32).T[128:200]),
        "b2T": np.asarray(inputs["b2"], np.float32).reshape(50, 1),
        "w3T": np.ascontiguousarray(np.asarray(inputs["w3"], np.float32).T),
        "b3T": np.asarray(inputs["b3"], np.float32).reshape(10, 1),
        "w4T": np.ascontiguousarray(np.asarray(inputs["w4"], np.float32).T),
        "b4T": np.asarray(inputs["b4"], np.float32).reshape(128, 1),
        "p8": np.tile(I16, (8, 1)).astype(np.float16),
        "P16": np.tile(I16, (1, 128)).astype(np.float16),
        "p8t": np.tile(I16, (1, 8)).astype(np.float16),
        "maskbd": np.kron(np.eye(8, dtype=np.float32),
                          np.ones((16, 16), np.float32)).astype(np.float16),
        "idn": np.eye(128, dtype=np.float32),
        "ones": np.ones((1, 512), np.float32),
        "ones16": np.ones((1, 512), np.float16),
        "ones128": np.ones((128, 1), np.float16),
    }
    for s in range(2):
        m[f"wa{s}"] = chunkT(np.ascontiguousarray(aw[s, 0].T)).astype(np.float16)
        m[f"wb{s}"] = chunkT(np.ascontiguousarray(aw[s, 1].T)).astype(np.float16)
        m[f"wf{s}"] = chunkT(np.ascontiguousarray(aw[s, 3].T)).astype(np.float16)
        m[f"ba{s}"] = np.ascontiguousarray(ab[s, 0].reshape(4, 128).T)
        m[f"bb{s}"] = np.ascontiguousarray(ab[s, 1].reshape(4, 128).T)
        m[f"bfC{s}"] = np.ascontiguousarray(ab[s, 3].reshape(4, 128).T)
        m[f"ba_r{s}"] = ab[s, 0].reshape(1, D1).astype(np.float16)
    return [m]


def kernel(**inputs):
    in_maps = make_in_maps(inputs)
    nc = build_program()
    res = run_bass_kernel_spmd(
        nc, in_maps, core_ids=[0],
        trace=bool(os.environ.get("KTRACE")))
    LAST["results"] = res
    return res.results[0]["outT"].T.astype(np.float32)
